# revision 14
# baseline (speedup 1.0000x reference)
# Trainium2 Bass kernel for nn_MultiHeadAttention (B=2, S=2048, D=1024, H=16).
#
# Sharding: head-tensor-parallel over 8 cores. Core c computes heads
# {2c, 2c+1}: column-sharded wq/wk/wv (128 output dims per core),
# row-sharded wo with the partial-output sum done on the host.
#
# Device layout strategy: Q/K live transposed (feature-dim on partitions) so
# the scores matmuls need no activation transposes:
#   QT/KT = (128 e_local, 4096 bs) computed with weight tiles stationary.
#   V is projected directly in natural (token, e) orientation — the token
#   subtile of the staged input chunk is the stationary operand — which is
#   exactly the layout the AV matmul needs (k on partitions). No transposes.
#   Scores are computed transposed, scoresT = (k, q): softmax exp runs
#   PSUM->SBUF on ACT and directly yields P^T for the AV matmul. Row-sums
#   come from a ones-column appended to V (the matmul computes them free).
#   Causal-boundary blocks are handled by multiplying P^T with a resident
#   0/1 triangle tile on DVE (f16 2x mode) — no bias matmuls.
# Bias handling: K bias dropped (softmax is invariant to per-query shifts),
#   V bias folded into the host-side output bias (softmax weights sum to 1:
#   out += wo @ bv), Q bias added on DVE as before.
# Dtypes: q/k/v stream + projections fp16, attention operands fp16, PSUM
# accumulation fp32, softmax exp + normalization fp32. Host sums the 8
# partial outputs in fp32 and adds bo + wo@bv.

import numpy as np

B, S, D, H = 2, 2048, 1024, 16
DK = D // H            # 64
NC = 8                 # cores
EL = D // NC           # 128 local e-dims (2 heads)
BS = B * S             # 4096 flattened tokens
NCH = 8                # projection bs-chunks of 512
CH = BS // NCH         # 512
NDT = D // 128         # 8 contraction tiles
NKT = S // 128         # 16 k-tiles per batch
NQB = S // 512         # 4 q-blocks per batch

SKIP, PLAIN = -1, -2   # block classes (>=0 means partial-pattern index)


def _classify_mask(mask):
    """Per (kt, qj) block classification of the (S_q, S_k) mask.

    Returns cls[kt][qj] (SKIP / PLAIN / pattern idx), rng[kt][qj] live col
    range, pr[kt][qj] partial col range, and the deduped 0/1 patterns
    (list of [128, w] float16 arrays) for the partial ranges."""
    m = np.asarray(mask).reshape(S, S)              # [q, k]; 0 = masked
    liveT = (m != 0).T                              # [k, q]
    cls = [[PLAIN] * NQB for _ in range(NKT)]
    rng = [[(0, 512)] * NQB for _ in range(NKT)]
    pr = [[(0, 0)] * NQB for _ in range(NKT)]
    uniq = {}
    pats = []
    for kt in range(NKT):
        for qj in range(NQB):
            blk = liveT[kt * 128:(kt + 1) * 128, qj * 512:(qj + 1) * 512]
            if blk.all():
                cls[kt][qj] = PLAIN
            elif not blk.any():
                cls[kt][qj] = SKIP
            else:
                live_col = blk.any(axis=0)
                nz = np.nonzero(live_col)[0]
                c0, c1 = int(nz[0]), int(nz[-1]) + 1
                rng[kt][qj] = (c0, c1)
                part_col = live_col & ~blk.all(axis=0)
                pz = np.nonzero(part_col)[0]
                p0, p1 = int(pz[0]), int(pz[-1]) + 1
                pr[kt][qj] = (p0, p1)
                pat = blk[:, p0:p1].astype(np.float16)
                key = (p1 - p0, pat.tobytes())
                if key not in uniq:
                    uniq[key] = len(pats)
                    pats.append(np.ascontiguousarray(pat))
                cls[kt][qj] = uniq[key]
    return cls, rng, pr, pats


def _build_program(cls, rng, pr, pat_widths):
    import concourse.bacc as bacc
    import concourse.mybir as mybir
    from concourse.tile import TileContext

    f32 = mybir.dt.float32
    f16 = mybir.dt.float16
    Exp = mybir.ActivationFunctionType.Exp
    mult = mybir.AluOpType.mult

    # pattern offsets inside the resident mask tile
    moff = []
    o = 0
    for w in pat_widths:
        moff.append(o)
        o += w
    MW = max(o, 1)

    nc = bacc.Bacc("TRN2", target_bir_lowering=False, debug=False,
                   num_devices=NC)

    qT = nc.dram_tensor("qT", [D, BS], f16, kind="ExternalInput")
    kT = nc.dram_tensor("kT", [D, BS], f16, kind="ExternalInput")
    vT = nc.dram_tensor("vT", [D, BS], f16, kind="ExternalInput")
    w3d = nc.dram_tensor("w3", [128, 3 * NDT * EL], f16,
                         kind="ExternalInput")
    woT = nc.dram_tensor("woT", [EL, D], f16, kind="ExternalInput")
    bqd = nc.dram_tensor("bq", [EL, 1], f32, kind="ExternalInput")
    maskd = nc.dram_tensor("masks", [128, MW], f16, kind="ExternalInput")
    out = nc.dram_tensor("out", [BS, D], f16, kind="ExternalOutput")

    # transposed-input views: [p, t, c] with t the 128-row block
    qT_r = qT.ap().rearrange("(t p) c -> p t c", p=128)
    kT_r = kT.ap().rearrange("(t p) c -> p t c", p=128)
    vT_r = vT.ap().rearrange("(t p) c -> p t c", p=128)
    w3_r = w3d.ap().rearrange("p (j t e) -> p j t e", j=3, t=NDT)

    with TileContext(nc) as tc:
        with (
            tc.tile_pool(name="const", bufs=1) as constp,
            tc.tile_pool(name="per", bufs=1) as perp,
            tc.tile_pool(name="stage", bufs=6) as stagep,
            tc.tile_pool(name="pt", bufs=12) as ptp,
            tc.tile_pool(name="zz", bufs=4) as zzp,
            tc.tile_pool(name="zb", bufs=8) as zbp,
            tc.tile_pool(name="ost", bufs=6) as ostp,
            tc.tile_pool(name="psA", bufs=2, space="PSUM") as psA,
            tc.tile_pool(name="psS", bufs=2, space="PSUM") as psS,
            tc.tile_pool(name="psO", bufs=2, space="PSUM") as psO,
        ):
            # ---- constants. w3 is split so the very first projection
            # matmuls can start after a ~200ns DMA ----
            w3 = constp.tile([128, 3, NDT, EL], f16, tag="w3")
            nc.sync.dma_start(out=w3[:, 0, 0:2], in_=w3_r[:, 0, 0:2])
            w_sb = {"q": w3[:, 0], "k": w3[:, 1], "v": w3[:, 2]}
            bq_sb = constp.tile([EL, 1], f32, tag="bq")
            msk = constp.tile([128, MW], f16, tag="msk")
            woT_sb = constp.tile([EL, D], f16, tag="wo")

            # const DMAs ride the ACT HWDGE queue so they never delay the
            # input-chunk stream on the SP queue
            def emit_w3_rest():
                nc.scalar.dma_start(out=w3[:, 0, 2:NDT],
                                    in_=w3_r[:, 0, 2:NDT])
                nc.scalar.dma_start(out=w3[:, 1:3], in_=w3_r[:, 1:3])
                nc.scalar.dma_start(out=bq_sb[:], in_=bqd.ap()[:])

            def emit_rest_consts():
                nc.scalar.dma_start(out=msk[:], in_=maskd.ap()[:])
                nc.scalar.dma_start(out=woT_sb[:], in_=woT.ap()[:])

            # ---- persistent activations ----
            QT_sb = perp.tile([EL, BS], f16, tag="QT")
            KT_sb = perp.tile([EL, BS], f16, tag="KT")
            OT_sb = perp.tile([EL, BS], f16, tag="OT")
            # V in natural (token, e) layout, packed per 128-row tile g as
            # 4 chunks of 64 cols: [V_A | ones | V_B | ones] so each head's
            # stationary operand [V_h | onescol] is one contiguous 128-col AP
            V_big = perp.tile([128, 4 * (BS // 128) * 64], f16, tag="Vb")
            V3 = V_big[:].rearrange("p (t x) -> p t x", x=64)

            def emit_vz():
                # zero the ones-chunks then set their first column to 1
                # (DVE: gpsimd.memset loses the value through walrus)
                nc.vector.memset(V3[:, 1::2, :], 0.0)
                nc.vector.memset(V3[:, 1::2, 0:1], 1.0)

            # ---- projections ----
            # Q/K: transposed layout, weight tiles stationary.
            # V: natural layout, token subtiles of the staged chunk
            # stationary, weight moving; lands directly in V_big.
            def emit_proj_chunk(j, name, src_r, dst, c, split=False):
                w = w_sb[name]  # AP view [p, t, e]
                st = stagep.tile([128, NDT, CH], f16, tag="stage")
                if split:
                    # first chunk of the kernel: land the first two d-tiles
                    # early so the PE starts before the full chunk arrives;
                    # the remaining weight DMAs go between the two pieces
                    # (they must be emitted before the matmuls that read
                    # them — dataflow deps don't order a later write)
                    nc.sync.dma_start(out=st[:, 0:2, :],
                                      in_=src_r[:, 0:2, c * CH:(c + 1) * CH])
                    emit_w3_rest()
                    nc.sync.dma_start(out=st[:, 2:NDT, :],
                                      in_=src_r[:, 2:NDT,
                                                c * CH:(c + 1) * CH])
                else:
                    nc.sync.dma_start(out=st[:],
                                      in_=src_r[:, :, c * CH:(c + 1) * CH])
                if name == "v":
                    ps = psA.tile([128, CH], f32, tag="proj")
                    for gg in range(CH // 128):
                        g = c * (CH // 128) + gg
                        sub = ps[:, gg * 128:(gg + 1) * 128]
                        for t in range(NDT):
                            nc.tensor.matmul(
                                sub, st[:, t, gg * 128:(gg + 1) * 128],
                                w[:, t, :],
                                start=(t == 0), stop=(t == NDT - 1))
                        nc.vector.tensor_copy(
                            V3[:, 4 * g:4 * g + 3:2, :],
                            sub.rearrange("p (a b) -> p a b", b=64))
                    return
                ps = psA.tile([EL, CH], f32, tag="proj")
                for t in range(NDT):
                    nc.tensor.matmul(ps[:], w[:, t, :], st[:, t, :],
                                     start=(t == 0), stop=(t == NDT - 1))
                if name == "q":
                    nc.vector.tensor_scalar_add(
                        dst[:, c * CH:(c + 1) * CH], ps[:], bq_sb[:])
                else:   # k: no bias (softmax shift-invariance)
                    nc.vector.tensor_copy(dst[:, c * CH:(c + 1) * CH], ps[:])

            JT = (("q", qT_r, QT_sb), ("k", kT_r, KT_sb), ("v", vT_r, None))

            # ---- attention ----
            def emit_attention(b, qjs, defer_oproj=False):
                for qj in qjs:
                    qlo = b * S + qj * 512
                    acts = [kt for kt in range(NKT) if cls[kt][qj] != SKIP]
                    if not acts:
                        continue
                    # both heads interleaved per kt-pair: PE always has the
                    # other head's matmuls while ACT runs this head's exp
                    ots = [psO.tile([128, 512], f32, tag="ot",
                                    name=f"ot{b}{qj}{hh}")
                           for hh in range(2)]
                    n_done = [0, 0]
                    for p0 in range(0, NKT, 2):
                        pair = [kt for kt in (p0, p0 + 1) if kt in acts]
                        if not pair:
                            continue
                        for h in range(2):
                            hs = slice(h * 64, (h + 1) * 64)
                            sc = psS.tile([128, 1024], f32, tag="score")
                            for kt in pair:
                                i = kt - p0
                                c0, c1 = rng[kt][qj]
                                klo = b * S + kt * 128
                                nc.tensor.matmul(
                                    sc[:, i * 512 + c0:i * 512 + c1],
                                    KT_sb[hs, klo:klo + 128],
                                    QT_sb[hs, qlo + c0:qlo + c1],
                                    start=True, stop=True)
                            pt = ptp.tile([128, 1024], f16, tag="pt")
                            spans = [(i * 512 + rng[kt][qj][0],
                                      i * 512 + rng[kt][qj][1])
                                     for kt in pair
                                     for i in [kt - p0]]
                            lo, hi = spans[0][0], spans[-1][1]
                            dead = (hi - lo) - sum(b - a for a, b in spans)
                            # one exp per pair unless the dead zone between
                            # the two kt spans is big enough to waste ACT
                            exp_spans = (spans if dead > 200
                                         else [(lo, hi)])
                            for a, bnd in exp_spans:
                                nc.scalar.activation(pt[:, a:bnd],
                                                     sc[:, a:bnd],
                                                     Exp, scale=0.125)
                            for kt in pair:
                                cl = cls[kt][qj]
                                if cl >= 0:
                                    # zero masked entries of P^T: cheap DVE
                                    # f16 multiply with the resident pattern
                                    i = kt - p0
                                    pp0, pp1 = pr[kt][qj]
                                    wdt = pp1 - pp0
                                    sl = slice(i * 512 + pp0, i * 512 + pp1)
                                    nc.vector.tensor_tensor(
                                        pt[:, sl], pt[:, sl],
                                        msk[:, moff[cl]:moff[cl] + wdt],
                                        op=mult)
                            for kt in pair:
                                i = kt - p0
                                c0, c1 = rng[kt][qj]
                                g = b * NKT + kt
                                vap = V_big[:, g * 256 + h * 128:
                                            g * 256 + (h + 1) * 128]
                                n_done[h] += 1
                                nc.tensor.matmul(
                                    ots[h][:, c0:c1], vap,
                                    pt[:, i * 512 + c0:i * 512 + c1],
                                    start=(n_done[h] == 1),
                                    stop=(n_done[h] == len(acts)))
                    for h in range(2):
                        # normalize: row 64 of ot = Z (sum of exp)
                        hs = slice(h * 64, (h + 1) * 64)
                        ot = ots[h]
                        z = zzp.tile([1, 512], f32, tag="z")
                        nc.vector.tensor_copy(z[:], ot[64:65, :])
                        rz = zzp.tile([1, 512], f32, tag="z")
                        nc.vector.reciprocal_approx_fast(rz[:], z[:])
                        rb = zbp.tile([64, 512], f32, tag="zb")
                        nc.gpsimd.partition_broadcast(rb[:], rz[:],
                                                      channels=64)
                        nc.vector.tensor_tensor(
                            OT_sb[hs, qlo:qlo + 512],
                            ot[0:64, :], rb[:], op=mult)
                    if not defer_oproj:
                        emit_oproj_qblock(b, qj)

            # ---- output projection (partial over local e-dims),
            # per q-block so outputs stream during attention. The two
            # PSUM->SBUF cast-copies split across DVE and ACT (Pool
            # cannot read PSUM in this backend) ----
            def emit_oproj_qblock(b, qj):
                for g in range(b * 16 + qj * 4, b * 16 + (qj + 1) * 4):
                    osr = ostp.tile([128, D], f16, tag="ost")
                    for j in range(2):
                        # po shares the projection PSUM ring: projections
                        # are finished by the tail, so the deferred oprojs
                        # get double-buffered PSUM instead of fighting the
                        # live ots tiles in psO
                        po = psA.tile([128, CH], f32, tag="proj")
                        nc.tensor.matmul(po[:],
                                         OT_sb[:, g * 128:(g + 1) * 128],
                                         woT_sb[:, j * 512:(j + 1) * 512],
                                         start=True, stop=True)
                        if j == 0:
                            nc.vector.tensor_copy(
                                osr[:, j * 512:(j + 1) * 512], po[:])
                        else:
                            nc.scalar.copy(
                                osr[:, j * 512:(j + 1) * 512], po[:])
                    nc.sync.dma_start(out=out.ap()[g * 128:(g + 1) * 128, :],
                                      in_=osr[:])

            # per-q-block interleave: chunk c feeds attention q-block c,
            # so projection matmuls act as PE filler while ACT runs exp.
            def emit_chunk3(c, first=False):
                for j, (name, src_r, dst) in enumerate(JT):
                    emit_proj_chunk(j, name, src_r, dst, c,
                                    split=(first and j == 0))
                    if first and j == 0:
                        emit_vz()
                    if first and j == 1:
                        emit_rest_consts()

            # batch 1 processes its smallest q-block (qj0, 4 k-tiles) LAST
            # so the final exp->AV->norm->oproj chain is as short as
            # possible; the deferred oprojs of qj2/qj3 fill the tail.
            for b in range(B):
                order = list(range(NQB)) if b == 0 else [1, 2, 3, 0]
                emitted = 0
                for qj in order:
                    # attention for this q-block may reach any k-tile the
                    # mask leaves active: emit every chunk it needs first
                    need = max([kt // 4 for kt in range(NKT)
                                if cls[kt][qj] != SKIP] + [qj])
                    while emitted <= need:
                        emit_chunk3(b * NQB + emitted,
                                    first=(b == 0 and emitted == 0))
                        emitted += 1
                    emit_attention(b, [qj],
                                   defer_oproj=(b == 1 and qj >= 2))
                if b == 1:
                    emit_oproj_qblock(1, 2)
                    emit_oproj_qblock(1, 3)
                while emitted < NQB:
                    emit_chunk3(b * NQB + emitted)
                    emitted += 1

    nc.compile()
    return nc


_CACHE = {}


def kernel(q, k, v, mask, wq, bq, wk, bk, wv, bv, wo, bo):
    from concourse.bass_utils import run_bass_kernel_spmd

    q = np.ascontiguousarray(np.asarray(q, np.float32).reshape(BS, D))
    k = np.ascontiguousarray(np.asarray(k, np.float32).reshape(BS, D))
    v = np.ascontiguousarray(np.asarray(v, np.float32).reshape(BS, D))
    wq = np.asarray(wq, np.float32)
    wk = np.asarray(wk, np.float32)
    wv = np.asarray(wv, np.float32)
    wo = np.asarray(wo, np.float32)
    bq = np.asarray(bq, np.float32)
    bv = np.asarray(bv, np.float32)
    bo = np.asarray(bo, np.float32)

    qTf = np.ascontiguousarray(q.T.astype(np.float16))
    kTf = np.ascontiguousarray(k.T.astype(np.float16))
    vTf = np.ascontiguousarray(v.T.astype(np.float16))

    cls, rng, pr, pats = _classify_mask(mask)
    pat_widths = [p.shape[1] for p in pats]
    key = (tuple(tuple(r) for r in cls), tuple(tuple(r) for r in rng),
           tuple(tuple(r) for r in pr), tuple(pat_widths))
    if key not in _CACHE:
        _CACHE[key] = _build_program(cls, rng, pr, pat_widths)
    nc = _CACHE[key]

    if pats:
        masks_np = np.ascontiguousarray(
            np.concatenate(pats, axis=1).astype(np.float16))
    else:
        masks_np = np.zeros((128, 1), np.float16)

    def pack_w3(c):
        el = slice(c * EL, (c + 1) * EL)
        ws = []
        for w in (wq, wk, wv):
            wt = np.ascontiguousarray(w[el, :].T.astype(np.float16))
            ws.append(wt.reshape(NDT, 128, EL).transpose(1, 0, 2))
        return np.ascontiguousarray(
            np.stack(ws, axis=1).reshape(128, 3 * NDT * EL))

    in_maps = []
    for c in range(NC):
        el = slice(c * EL, (c + 1) * EL)
        m = {
            "qT": qTf, "kT": kTf, "vT": vTf,
            "w3": pack_w3(c),
            "woT": np.ascontiguousarray(wo[:, el].T.astype(np.float16)),
            "bq": np.ascontiguousarray(bq[el][:, None]),
            "masks": masks_np,
        }
        in_maps.append(m)

    res = run_bass_kernel_spmd(nc, in_maps, list(range(NC)))
    acc = res.results[0]["out"].astype(np.float32)
    for c in range(1, NC):
        acc = acc + res.results[c]["out"]
    # bo plus the folded V bias: softmax weights sum to 1 so the V bias
    # contributes wo @ bv to every output row
    acc = acc + (bo + wo @ bv)[None, :]
    return acc.reshape(B, S, D)


# revision 16
# speedup vs baseline: 1.3246x; 1.3246x over previous
# Trainium2 Bass kernel for nn_MultiHeadAttention (B=2, S=2048, D=1024, H=16).
#
# Sharding: head-tensor-parallel over 8 cores. Core c computes heads
# {2c, 2c+1}: column-sharded wq/wk/wv (128 output dims per core),
# row-sharded wo with the partial-output sum done on the host.
#
# Device layout strategy: Q/K live transposed (feature-dim on partitions) so
# the scores matmuls need no activation transposes:
#   QT/KT = (128 e_local, 4096 bs) computed with weight tiles stationary.
#   V is projected directly in natural (token, e) orientation — the token
#   subtile of the staged input chunk is the stationary operand — which is
#   exactly the layout the AV matmul needs (k on partitions). No transposes.
#   Scores are computed transposed, scoresT = (k, q): softmax exp runs
#   PSUM->SBUF on ACT and directly yields P^T for the AV matmul. Row-sums
#   come from a ones-column appended to V (the matmul computes them free).
#   Causal-boundary blocks are handled by multiplying P^T with a resident
#   0/1 triangle tile on DVE (f16 2x mode) — no bias matmuls.
# Bias handling: K bias dropped (softmax is invariant to per-query shifts),
#   V bias folded into the host-side output bias (softmax weights sum to 1:
#   out += wo @ bv), Q bias added on DVE as before.
# Dtypes: q/k/v stream + projections fp16, attention operands fp16, PSUM
# accumulation fp32, softmax exp + normalization fp32. Host sums the 8
# partial outputs in fp32 and adds bo + wo@bv.

import numpy as np

B, S, D, H = 2, 2048, 1024, 16
DK = D // H            # 64
NC = 8                 # cores
EL = D // NC           # 128 local e-dims (2 heads)
BS = B * S             # 4096 flattened tokens
NCH = 8                # projection bs-chunks of 512
CH = BS // NCH         # 512
NDT = D // 128         # 8 contraction tiles
NKT = S // 128         # 16 k-tiles per batch
NQB = S // 512         # 4 q-blocks per batch

SKIP, PLAIN = -1, -2   # block classes (>=0 means partial-pattern index)


def _classify_mask(mask):
    """Per (kt, qj) block classification of the (S_q, S_k) mask.

    Returns cls[kt][qj] (SKIP / PLAIN / pattern idx), rng[kt][qj] live col
    range, pr[kt][qj] partial col range, and the deduped 0/1 patterns
    (list of [128, w] float16 arrays) for the partial ranges."""
    m = np.asarray(mask).reshape(S, S)              # [q, k]; 0 = masked
    liveT = (m != 0).T                              # [k, q]
    cls = [[PLAIN] * NQB for _ in range(NKT)]
    rng = [[(0, 512)] * NQB for _ in range(NKT)]
    pr = [[(0, 0)] * NQB for _ in range(NKT)]
    uniq = {}
    pats = []
    for kt in range(NKT):
        for qj in range(NQB):
            blk = liveT[kt * 128:(kt + 1) * 128, qj * 512:(qj + 1) * 512]
            if blk.all():
                cls[kt][qj] = PLAIN
            elif not blk.any():
                cls[kt][qj] = SKIP
            else:
                live_col = blk.any(axis=0)
                nz = np.nonzero(live_col)[0]
                c0, c1 = int(nz[0]), int(nz[-1]) + 1
                rng[kt][qj] = (c0, c1)
                part_col = live_col & ~blk.all(axis=0)
                pz = np.nonzero(part_col)[0]
                p0, p1 = int(pz[0]), int(pz[-1]) + 1
                pr[kt][qj] = (p0, p1)
                pat = blk[:, p0:p1].astype(np.float16)
                key = (p1 - p0, pat.tobytes())
                if key not in uniq:
                    uniq[key] = len(pats)
                    pats.append(np.ascontiguousarray(pat))
                cls[kt][qj] = uniq[key]
    return cls, rng, pr, pats


def _build_program(cls, rng, pr, pat_widths):
    import concourse.bacc as bacc
    import concourse.mybir as mybir
    from concourse.tile import TileContext

    f32 = mybir.dt.float32
    f16 = mybir.dt.float16
    Exp = mybir.ActivationFunctionType.Exp
    mult = mybir.AluOpType.mult

    # pattern offsets inside the resident mask tile
    moff = []
    o = 0
    for w in pat_widths:
        moff.append(o)
        o += w
    MW = max(o, 1)

    nc = bacc.Bacc("TRN2", target_bir_lowering=False, debug=False,
                   num_devices=NC)

    qT = nc.dram_tensor("qT", [D, BS], f16, kind="ExternalInput")
    kT = nc.dram_tensor("kT", [D, BS], f16, kind="ExternalInput")
    vT = nc.dram_tensor("vT", [D, BS], f16, kind="ExternalInput")
    w3d = nc.dram_tensor("w3", [128, 3 * NDT * EL], f16,
                         kind="ExternalInput")
    woT = nc.dram_tensor("woT", [EL, D], f16, kind="ExternalInput")
    bqd = nc.dram_tensor("bq", [EL, 1], f32, kind="ExternalInput")
    maskd = nc.dram_tensor("masks", [128, MW], f16, kind="ExternalInput")
    out = nc.dram_tensor("out", [BS, D], f16, kind="ExternalOutput")

    # transposed-input views: [p, t, c] with t the 128-row block
    qT_r = qT.ap().rearrange("(t p) c -> p t c", p=128)
    kT_r = kT.ap().rearrange("(t p) c -> p t c", p=128)
    vT_r = vT.ap().rearrange("(t p) c -> p t c", p=128)
    w3_r = w3d.ap().rearrange("p (j t e) -> p j t e", j=3, t=NDT)

    with TileContext(nc) as tc:
        with (
            tc.tile_pool(name="const", bufs=1) as constp,
            tc.tile_pool(name="per", bufs=1) as perp,
            tc.tile_pool(name="stage", bufs=6) as stagep,
            tc.tile_pool(name="pt", bufs=12) as ptp,
            tc.tile_pool(name="zz", bufs=4) as zzp,
            tc.tile_pool(name="zb", bufs=8) as zbp,
            tc.tile_pool(name="ost", bufs=6) as ostp,
            tc.tile_pool(name="psA", bufs=2, space="PSUM") as psA,
            tc.tile_pool(name="psS", bufs=2, space="PSUM") as psS,
            tc.tile_pool(name="psO", bufs=2, space="PSUM") as psO,
        ):
            # ---- constants. w3 is split so the very first projection
            # matmuls can start after a ~200ns DMA ----
            w3 = constp.tile([128, 3, NDT, EL], f16, tag="w3")
            nc.sync.dma_start(out=w3[:, 0, 0:2], in_=w3_r[:, 0, 0:2])
            w_sb = {"q": w3[:, 0], "k": w3[:, 1], "v": w3[:, 2]}
            bq_sb = constp.tile([EL, 1], f32, tag="bq")
            msk = constp.tile([128, MW], f16, tag="msk")
            woT_sb = constp.tile([EL, D], f16, tag="wo")

            # const DMAs ride the ACT HWDGE queue so they never delay the
            # input-chunk stream on the SP queue
            def emit_w3_rest():
                nc.scalar.dma_start(out=w3[:, 0, 2:NDT],
                                    in_=w3_r[:, 0, 2:NDT])
                nc.scalar.dma_start(out=w3[:, 1:3], in_=w3_r[:, 1:3])
                nc.scalar.dma_start(out=bq_sb[:], in_=bqd.ap()[:])

            def emit_rest_consts():
                nc.scalar.dma_start(out=msk[:], in_=maskd.ap()[:])
                nc.scalar.dma_start(out=woT_sb[:], in_=woT.ap()[:])

            # ---- persistent activations ----
            QT_sb = perp.tile([EL, BS], f16, tag="QT")
            KT_sb = perp.tile([EL, BS], f16, tag="KT")
            OT_sb = perp.tile([EL, BS], f16, tag="OT")
            # V in natural (token, e) layout, packed per 128-row tile g as
            # 4 chunks of 64 cols: [V_A | ones | V_B | ones] so each head's
            # stationary operand [V_h | onescol] is one contiguous 128-col AP
            V_big = perp.tile([128, 4 * (BS // 128) * 64], f16, tag="Vb")
            V3 = V_big[:].rearrange("p (t x) -> p t x", x=64)

            def emit_vz():
                # zero the ones-chunks then set their first column to 1
                # (DVE: gpsimd.memset loses the value through walrus)
                nc.vector.memset(V3[:, 1::2, :], 0.0)
                nc.vector.memset(V3[:, 1::2, 0:1], 1.0)

            # ---- projections ----
            # Q/K: transposed layout, weight tiles stationary.
            # V: natural layout, token subtiles of the staged chunk
            # stationary, weight moving; lands directly in V_big.
            def emit_proj_chunk(j, name, src_r, dst, c, split=False):
                w = w_sb[name]  # AP view [p, t, e]
                st = stagep.tile([128, NDT, CH], f16, tag="stage")
                if split:
                    # first chunk of the kernel: land the first two d-tiles
                    # early so the PE starts before the full chunk arrives;
                    # the remaining weight DMAs go between the two pieces
                    # (they must be emitted before the matmuls that read
                    # them — dataflow deps don't order a later write)
                    nc.sync.dma_start(out=st[:, 0:2, :],
                                      in_=src_r[:, 0:2, c * CH:(c + 1) * CH])
                    emit_w3_rest()
                    nc.sync.dma_start(out=st[:, 2:NDT, :],
                                      in_=src_r[:, 2:NDT,
                                                c * CH:(c + 1) * CH])
                else:
                    nc.sync.dma_start(out=st[:],
                                      in_=src_r[:, :, c * CH:(c + 1) * CH])
                if name == "v":
                    ps = psA.tile([128, CH], f32, tag="proj")
                    for gg in range(CH // 128):
                        g = c * (CH // 128) + gg
                        sub = ps[:, gg * 128:(gg + 1) * 128]
                        for t in range(NDT):
                            nc.tensor.matmul(
                                sub, st[:, t, gg * 128:(gg + 1) * 128],
                                w[:, t, :],
                                start=(t == 0), stop=(t == NDT - 1))
                        nc.vector.tensor_copy(
                            V3[:, 4 * g:4 * g + 3:2, :],
                            sub.rearrange("p (a b) -> p a b", b=64))
                    return
                ps = psA.tile([EL, CH], f32, tag="proj")
                for t in range(NDT):
                    nc.tensor.matmul(ps[:], w[:, t, :], st[:, t, :],
                                     start=(t == 0), stop=(t == NDT - 1))
                if name == "q":
                    nc.vector.tensor_scalar_add(
                        dst[:, c * CH:(c + 1) * CH], ps[:], bq_sb[:])
                else:   # k: no bias (softmax shift-invariance)
                    nc.vector.tensor_copy(dst[:, c * CH:(c + 1) * CH], ps[:])

            JT = (("q", qT_r, QT_sb), ("k", kT_r, KT_sb), ("v", vT_r, None))

            # ---- attention ----
            def emit_attention(b, qjs, defer_oproj=False):
                for qj in qjs:
                    qlo = b * S + qj * 512
                    acts = [kt for kt in range(NKT) if cls[kt][qj] != SKIP]
                    if not acts:
                        continue
                    # both heads interleaved per kt-pair: PE always has the
                    # other head's matmuls while ACT runs this head's exp
                    ots = [psO.tile([128, 512], f32, tag="ot",
                                    name=f"ot{b}{qj}{hh}")
                           for hh in range(2)]
                    n_done = [0, 0]
                    for p0 in range(0, NKT, 2):
                        pair = [kt for kt in (p0, p0 + 1) if kt in acts]
                        if not pair:
                            continue
                        for h in range(2):
                            hs = slice(h * 64, (h + 1) * 64)
                            sc = psS.tile([128, 1024], f32, tag="score")
                            for kt in pair:
                                i = kt - p0
                                c0, c1 = rng[kt][qj]
                                klo = b * S + kt * 128
                                nc.tensor.matmul(
                                    sc[:, i * 512 + c0:i * 512 + c1],
                                    KT_sb[hs, klo:klo + 128],
                                    QT_sb[hs, qlo + c0:qlo + c1],
                                    start=True, stop=True)
                            pt = ptp.tile([128, 1024], f16, tag="pt")
                            spans = [(i * 512 + rng[kt][qj][0],
                                      i * 512 + rng[kt][qj][1])
                                     for kt in pair
                                     for i in [kt - p0]]
                            lo, hi = spans[0][0], spans[-1][1]
                            dead = (hi - lo) - sum(b - a for a, b in spans)
                            # one exp per pair unless the dead zone between
                            # the two kt spans is big enough to waste ACT
                            exp_spans = (spans if dead > 200
                                         else [(lo, hi)])
                            for a, bnd in exp_spans:
                                nc.scalar.activation(pt[:, a:bnd],
                                                     sc[:, a:bnd],
                                                     Exp, scale=0.125)
                            for kt in pair:
                                cl = cls[kt][qj]
                                if cl >= 0:
                                    # zero masked entries of P^T: cheap DVE
                                    # f16 multiply with the resident pattern
                                    i = kt - p0
                                    pp0, pp1 = pr[kt][qj]
                                    wdt = pp1 - pp0
                                    sl = slice(i * 512 + pp0, i * 512 + pp1)
                                    nc.vector.tensor_tensor(
                                        pt[:, sl], pt[:, sl],
                                        msk[:, moff[cl]:moff[cl] + wdt],
                                        op=mult)
                            for kt in pair:
                                i = kt - p0
                                c0, c1 = rng[kt][qj]
                                g = b * NKT + kt
                                vap = V_big[:, g * 256 + h * 128:
                                            g * 256 + (h + 1) * 128]
                                n_done[h] += 1
                                nc.tensor.matmul(
                                    ots[h][:, c0:c1], vap,
                                    pt[:, i * 512 + c0:i * 512 + c1],
                                    start=(n_done[h] == 1),
                                    stop=(n_done[h] == len(acts)))
                    for h in range(2):
                        # normalize: row 64 of ot = Z (sum of exp)
                        hs = slice(h * 64, (h + 1) * 64)
                        ot = ots[h]
                        z = zzp.tile([1, 512], f32, tag="z")
                        nc.vector.tensor_copy(z[:], ot[64:65, :])
                        rz = zzp.tile([1, 512], f32, tag="z")
                        nc.vector.reciprocal_approx_fast(rz[:], z[:])
                        rb = zbp.tile([64, 512], f32, tag="zb")
                        nc.gpsimd.partition_broadcast(rb[:], rz[:],
                                                      channels=64)
                        nc.vector.tensor_tensor(
                            OT_sb[hs, qlo:qlo + 512],
                            ot[0:64, :], rb[:], op=mult)
                    if not defer_oproj:
                        emit_oproj_qblock(b, qj)

            # ---- output projection (partial over local e-dims),
            # per q-block so outputs stream during attention. The two
            # PSUM->SBUF cast-copies split across DVE and ACT (Pool
            # cannot read PSUM in this backend) ----
            def emit_oproj_qblock(b, qj, tail=False):
                for g in range(b * 16 + qj * 4, b * 16 + (qj + 1) * 4):
                    osr = ostp.tile([128, D], f16, tag="ost")
                    for j in range(2):
                        # tail oprojs borrow the projection PSUM ring
                        # (projections are done by then) so they get
                        # double-buffered PSUM instead of fighting the
                        # live ots tiles in psO
                        if tail:
                            po = psA.tile([128, CH], f32, tag="proj")
                        else:
                            po = psO.tile([128, 512], f32, tag="ot")
                        nc.tensor.matmul(po[:],
                                         OT_sb[:, g * 128:(g + 1) * 128],
                                         woT_sb[:, j * 512:(j + 1) * 512],
                                         start=True, stop=True)
                        if j == 0:
                            nc.vector.tensor_copy(
                                osr[:, j * 512:(j + 1) * 512], po[:])
                        else:
                            nc.scalar.copy(
                                osr[:, j * 512:(j + 1) * 512], po[:])
                    nc.sync.dma_start(out=out.ap()[g * 128:(g + 1) * 128, :],
                                      in_=osr[:])

            # per-q-block interleave: chunk c feeds attention q-block c,
            # so projection matmuls act as PE filler while ACT runs exp.
            def emit_chunk3(c, first=False):
                for j, (name, src_r, dst) in enumerate(JT):
                    emit_proj_chunk(j, name, src_r, dst, c,
                                    split=(first and j == 0))
                    if first and j == 0:
                        emit_vz()
                    if first and j == 1:
                        emit_rest_consts()

            # batch 1 processes its smallest q-block (qj0, 4 k-tiles) LAST
            # so the final exp->AV->norm->oproj chain is as short as
            # possible; the deferred oprojs of qj2/qj3 fill the tail.
            for b in range(B):
                order = list(range(NQB)) if b == 0 else [1, 2, 3, 0]
                emitted = 0
                for qj in order:
                    # attention for this q-block may reach any k-tile the
                    # mask leaves active: emit every chunk it needs first
                    need = max([kt // 4 for kt in range(NKT)
                                if cls[kt][qj] != SKIP] + [qj])
                    while emitted <= need:
                        emit_chunk3(b * NQB + emitted,
                                    first=(b == 0 and emitted == 0))
                        emitted += 1
                    emit_attention(b, [qj],
                                   defer_oproj=(b == 1 and qj >= 2))
                if b == 1:
                    emit_oproj_qblock(1, 2, tail=True)
                    emit_oproj_qblock(1, 3, tail=True)
                while emitted < NQB:
                    emit_chunk3(b * NQB + emitted)
                    emitted += 1

    nc.compile()
    return nc


_CACHE = {}


def kernel(q, k, v, mask, wq, bq, wk, bk, wv, bv, wo, bo):
    from concourse.bass_utils import run_bass_kernel_spmd

    q = np.ascontiguousarray(np.asarray(q, np.float32).reshape(BS, D))
    k = np.ascontiguousarray(np.asarray(k, np.float32).reshape(BS, D))
    v = np.ascontiguousarray(np.asarray(v, np.float32).reshape(BS, D))
    wq = np.asarray(wq, np.float32)
    wk = np.asarray(wk, np.float32)
    wv = np.asarray(wv, np.float32)
    wo = np.asarray(wo, np.float32)
    bq = np.asarray(bq, np.float32)
    bv = np.asarray(bv, np.float32)
    bo = np.asarray(bo, np.float32)

    qTf = np.ascontiguousarray(q.T.astype(np.float16))
    kTf = np.ascontiguousarray(k.T.astype(np.float16))
    vTf = np.ascontiguousarray(v.T.astype(np.float16))

    cls, rng, pr, pats = _classify_mask(mask)
    pat_widths = [p.shape[1] for p in pats]
    key = (tuple(tuple(r) for r in cls), tuple(tuple(r) for r in rng),
           tuple(tuple(r) for r in pr), tuple(pat_widths))
    if key not in _CACHE:
        _CACHE[key] = _build_program(cls, rng, pr, pat_widths)
    nc = _CACHE[key]

    if pats:
        masks_np = np.ascontiguousarray(
            np.concatenate(pats, axis=1).astype(np.float16))
    else:
        masks_np = np.zeros((128, 1), np.float16)

    def pack_w3(c):
        el = slice(c * EL, (c + 1) * EL)
        ws = []
        for w in (wq, wk, wv):
            wt = np.ascontiguousarray(w[el, :].T.astype(np.float16))
            ws.append(wt.reshape(NDT, 128, EL).transpose(1, 0, 2))
        return np.ascontiguousarray(
            np.stack(ws, axis=1).reshape(128, 3 * NDT * EL))

    in_maps = []
    for c in range(NC):
        el = slice(c * EL, (c + 1) * EL)
        m = {
            "qT": qTf, "kT": kTf, "vT": vTf,
            "w3": pack_w3(c),
            "woT": np.ascontiguousarray(wo[:, el].T.astype(np.float16)),
            "bq": np.ascontiguousarray(bq[el][:, None]),
            "masks": masks_np,
        }
        in_maps.append(m)

    res = run_bass_kernel_spmd(nc, in_maps, list(range(NC)))
    acc = res.results[0]["out"].astype(np.float32)
    for c in range(1, NC):
        acc = acc + res.results[c]["out"]
    # bo plus the folded V bias: softmax weights sum to 1 so the V bias
    # contributes wo @ bv to every output row
    acc = acc + (bo + wo @ bv)[None, :]
    return acc.reshape(B, S, D)


# revision 20
# speedup vs baseline: 1.3491x; 1.0185x over previous
# Trainium2 Bass kernel for nn_MultiHeadAttention (B=2, S=2048, D=1024, H=16).
#
# Sharding: head-tensor-parallel over 8 cores. Core c computes heads
# {2c, 2c+1}: column-sharded wq/wk/wv (128 output dims per core),
# row-sharded wo with the partial-output sum done on the host.
#
# Device layout strategy: Q/K live transposed (feature-dim on partitions) so
# the scores matmuls need no activation transposes:
#   QT/KT = (128 e_local, 4096 bs) computed with weight tiles stationary.
#   V is projected directly in natural (token, e) orientation — the token
#   subtile of the staged input chunk is the stationary operand — which is
#   exactly the layout the AV matmul needs (k on partitions). No transposes.
#   Scores are computed transposed, scoresT = (k, q): softmax exp runs
#   PSUM->SBUF on ACT and directly yields P^T for the AV matmul. Row-sums
#   come from a ones-column appended to V (the matmul computes them free).
#   Causal-boundary blocks are handled by multiplying P^T with a resident
#   0/1 triangle tile on DVE (f16 2x mode) — no bias matmuls.
# Bias handling: K bias dropped (softmax is invariant to per-query shifts),
#   V bias folded into the host-side output bias (softmax weights sum to 1:
#   out += wo @ bv), Q bias added on DVE as before.
# Dtypes: q/k/v stream + projections fp16, attention operands fp16, PSUM
# accumulation fp32, softmax exp + normalization fp32. Host sums the 8
# partial outputs in fp32 and adds bo + wo@bv.

import numpy as np

B, S, D, H = 2, 2048, 1024, 16
DK = D // H            # 64
NC = 8                 # cores
EL = D // NC           # 128 local e-dims (2 heads)
BS = B * S             # 4096 flattened tokens
NCH = 8                # projection bs-chunks of 512
CH = BS // NCH         # 512
NDT = D // 128         # 8 contraction tiles
NKT = S // 128         # 16 k-tiles per batch
NQB = S // 512         # 4 q-blocks per batch

SKIP, PLAIN = -1, -2   # block classes (>=0 means partial-pattern index)


def _classify_mask(mask):
    """Per (kt, qj) block classification of the (S_q, S_k) mask.

    Returns cls[kt][qj] (SKIP / PLAIN / pattern idx), rng[kt][qj] live col
    range, pr[kt][qj] partial col range, and the deduped 0/1 patterns
    (list of [128, w] float16 arrays) for the partial ranges."""
    m = np.asarray(mask).reshape(S, S)              # [q, k]; 0 = masked
    liveT = (m != 0).T                              # [k, q]
    cls = [[PLAIN] * NQB for _ in range(NKT)]
    rng = [[(0, 512)] * NQB for _ in range(NKT)]
    pr = [[(0, 0)] * NQB for _ in range(NKT)]
    uniq = {}
    pats = []
    for kt in range(NKT):
        for qj in range(NQB):
            blk = liveT[kt * 128:(kt + 1) * 128, qj * 512:(qj + 1) * 512]
            if blk.all():
                cls[kt][qj] = PLAIN
            elif not blk.any():
                cls[kt][qj] = SKIP
            else:
                live_col = blk.any(axis=0)
                nz = np.nonzero(live_col)[0]
                c0, c1 = int(nz[0]), int(nz[-1]) + 1
                rng[kt][qj] = (c0, c1)
                part_col = live_col & ~blk.all(axis=0)
                pz = np.nonzero(part_col)[0]
                p0, p1 = int(pz[0]), int(pz[-1]) + 1
                pr[kt][qj] = (p0, p1)
                pat = blk[:, p0:p1].astype(np.float16)
                key = (p1 - p0, pat.tobytes())
                if key not in uniq:
                    uniq[key] = len(pats)
                    pats.append(np.ascontiguousarray(pat))
                cls[kt][qj] = uniq[key]
    return cls, rng, pr, pats


def _build_program(cls, rng, pr, pat_widths):
    import concourse.bacc as bacc
    import concourse.mybir as mybir
    from concourse.tile import TileContext

    f32 = mybir.dt.float32
    f16 = mybir.dt.float16
    Exp = mybir.ActivationFunctionType.Exp
    mult = mybir.AluOpType.mult

    # pattern offsets inside the resident mask tile
    moff = []
    o = 0
    for w in pat_widths:
        moff.append(o)
        o += w
    MW = max(o, 1)

    nc = bacc.Bacc("TRN2", target_bir_lowering=False, debug=False,
                   num_devices=NC)

    qT = nc.dram_tensor("qT", [D, BS], f16, kind="ExternalInput")
    kT = nc.dram_tensor("kT", [D, BS], f16, kind="ExternalInput")
    vT = nc.dram_tensor("vT", [D, BS], f16, kind="ExternalInput")
    w3d = nc.dram_tensor("w3", [128, 3 * NDT * EL], f16,
                         kind="ExternalInput")
    woT = nc.dram_tensor("woT", [EL, D], f16, kind="ExternalInput")
    bqd = nc.dram_tensor("bq", [EL, 1], f32, kind="ExternalInput")
    maskd = nc.dram_tensor("masks", [128, MW], f16, kind="ExternalInput")
    out = nc.dram_tensor("out", [BS, D], f16, kind="ExternalOutput")

    # transposed-input views: [p, t, c] with t the 128-row block
    qT_r = qT.ap().rearrange("(t p) c -> p t c", p=128)
    kT_r = kT.ap().rearrange("(t p) c -> p t c", p=128)
    vT_r = vT.ap().rearrange("(t p) c -> p t c", p=128)
    w3_r = w3d.ap().rearrange("p (j t e) -> p j t e", j=3, t=NDT)

    with TileContext(nc) as tc:
        with (
            tc.tile_pool(name="const", bufs=1) as constp,
            tc.tile_pool(name="per", bufs=1) as perp,
            tc.tile_pool(name="stage", bufs=6) as stagep,
            tc.tile_pool(name="pt", bufs=12) as ptp,
            tc.tile_pool(name="zz", bufs=4) as zzp,
            tc.tile_pool(name="zb", bufs=8) as zbp,
            tc.tile_pool(name="ost", bufs=6) as ostp,
            tc.tile_pool(name="psA", bufs=2, space="PSUM") as psA,
            tc.tile_pool(name="psS", bufs=2, space="PSUM") as psS,
            tc.tile_pool(name="psO", bufs=2, space="PSUM") as psO,
        ):
            # ---- constants. w3 is split so the very first projection
            # matmuls can start after a ~200ns DMA ----
            w3 = constp.tile([128, 3, NDT, EL], f16, tag="w3")
            nc.sync.dma_start(out=w3[:, 0, 0:2], in_=w3_r[:, 0, 0:2])
            w_sb = {"q": w3[:, 0], "k": w3[:, 1], "v": w3[:, 2]}
            bq_sb = constp.tile([EL, 1], f32, tag="bq")
            msk = constp.tile([128, MW], f16, tag="msk")
            woT_sb = constp.tile([EL, D], f16, tag="wo")

            # w3-rest stays on the SP queue AFTER the first staged input
            # pieces: same-queue FIFO keeps it from grabbing the DMA
            # device ahead of the critical first chunk
            def emit_w3_rest():
                nc.sync.dma_start(out=w3[:, 0, 2:NDT], in_=w3_r[:, 0, 2:NDT])
                nc.sync.dma_start(out=w3[:, 1:3], in_=w3_r[:, 1:3])
                nc.sync.dma_start(out=bq_sb[:], in_=bqd.ap()[:])

            # masks are tiny (jumping the queue is harmless); woT rides
            # the ACT HWDGE queue so it doesn't delay input chunks
            def emit_rest_consts():
                nc.scalar.dma_start(out=msk[:], in_=maskd.ap()[:])
                nc.scalar.dma_start(out=woT_sb[:], in_=woT.ap()[:])

            # ---- persistent activations ----
            QT_sb = perp.tile([EL, BS], f16, tag="QT")
            KT_sb = perp.tile([EL, BS], f16, tag="KT")
            OT_sb = perp.tile([EL, BS], f16, tag="OT")
            # V in natural (token, e) layout, packed per 128-row tile g as
            # 4 chunks of 64 cols: [V_A | ones | V_B | ones] so each head's
            # stationary operand [V_h | onescol] is one contiguous 128-col AP
            V_big = perp.tile([128, 4 * (BS // 128) * 64], f16, tag="Vb")
            V3 = V_big[:].rearrange("p (t x) -> p t x", x=64)

            def emit_vz():
                # zero the ones-chunks then set their first column to 1
                # (DVE: gpsimd.memset loses the value through walrus)
                nc.vector.memset(V3[:, 1::2, :], 0.0)
                nc.vector.memset(V3[:, 1::2, 0:1], 1.0)

            # ---- projections ----
            # Q/K: transposed layout, weight tiles stationary.
            # V: natural layout, token subtiles of the staged chunk
            # stationary, weight moving; lands directly in V_big.
            def emit_proj_chunk(j, name, src_r, dst, c, split=False):
                w = w_sb[name]  # AP view [p, t, e]
                st = stagep.tile([128, NDT, CH], f16, tag="stage")
                if split:
                    # first chunk of the kernel: land the first two d-tiles
                    # early so the PE starts before the full chunk arrives;
                    # the remaining weight DMAs go between the two pieces
                    # (they must be emitted before the matmuls that read
                    # them — dataflow deps don't order a later write)
                    nc.sync.dma_start(out=st[:, 0:2, :],
                                      in_=src_r[:, 0:2, c * CH:(c + 1) * CH])
                    nc.sync.dma_start(out=st[:, 2:NDT, :],
                                      in_=src_r[:, 2:NDT,
                                                c * CH:(c + 1) * CH])
                    emit_w3_rest()
                else:
                    nc.sync.dma_start(out=st[:],
                                      in_=src_r[:, :, c * CH:(c + 1) * CH])
                if name == "v":
                    ps = psA.tile([128, CH], f32, tag="proj")
                    for gg in range(CH // 128):
                        g = c * (CH // 128) + gg
                        sub = ps[:, gg * 128:(gg + 1) * 128]
                        for t in range(NDT):
                            nc.tensor.matmul(
                                sub, st[:, t, gg * 128:(gg + 1) * 128],
                                w[:, t, :],
                                start=(t == 0), stop=(t == NDT - 1))
                        nc.vector.tensor_copy(
                            V3[:, 4 * g:4 * g + 3:2, :],
                            sub.rearrange("p (a b) -> p a b", b=64))
                    return
                ps = psA.tile([EL, CH], f32, tag="proj")
                for t in range(NDT):
                    nc.tensor.matmul(ps[:], w[:, t, :], st[:, t, :],
                                     start=(t == 0), stop=(t == NDT - 1))
                if name == "q":
                    nc.vector.tensor_scalar_add(
                        dst[:, c * CH:(c + 1) * CH], ps[:], bq_sb[:])
                else:   # k: no bias (softmax shift-invariance)
                    nc.vector.tensor_copy(dst[:, c * CH:(c + 1) * CH], ps[:])

            JT = (("q", qT_r, QT_sb), ("k", kT_r, KT_sb), ("v", vT_r, None))

            # ---- attention ----
            def emit_attention(b, qjs, defer_oproj=False):
                for qj in qjs:
                    qlo = b * S + qj * 512
                    acts = [kt for kt in range(NKT) if cls[kt][qj] != SKIP]
                    if not acts:
                        continue
                    # both heads interleaved per kt-pair: PE always has the
                    # other head's matmuls while ACT runs this head's exp
                    ots = [psO.tile([128, 512], f32, tag="ot",
                                    name=f"ot{b}{qj}{hh}")
                           for hh in range(2)]
                    n_done = [0, 0]
                    for p0 in range(0, NKT, 2):
                        pair = [kt for kt in (p0, p0 + 1) if kt in acts]
                        if not pair:
                            continue
                        for h in range(2):
                            hs = slice(h * 64, (h + 1) * 64)
                            sc = psS.tile([128, 1024], f32, tag="score")
                            for kt in pair:
                                i = kt - p0
                                c0, c1 = rng[kt][qj]
                                klo = b * S + kt * 128
                                nc.tensor.matmul(
                                    sc[:, i * 512 + c0:i * 512 + c1],
                                    KT_sb[hs, klo:klo + 128],
                                    QT_sb[hs, qlo + c0:qlo + c1],
                                    start=True, stop=True)
                            pt = ptp.tile([128, 1024], f16, tag="pt")
                            spans = [(i * 512 + rng[kt][qj][0],
                                      i * 512 + rng[kt][qj][1])
                                     for kt in pair
                                     for i in [kt - p0]]
                            lo, hi = spans[0][0], spans[-1][1]
                            dead = (hi - lo) - sum(b - a for a, b in spans)
                            # one exp per pair unless the dead zone between
                            # the two kt spans is big enough to waste ACT
                            exp_spans = (spans if dead > 200
                                         else [(lo, hi)])
                            for a, bnd in exp_spans:
                                nc.scalar.activation(pt[:, a:bnd],
                                                     sc[:, a:bnd],
                                                     Exp, scale=0.125)
                            for kt in pair:
                                cl = cls[kt][qj]
                                if cl >= 0:
                                    # zero masked entries of P^T: cheap DVE
                                    # f16 multiply with the resident pattern
                                    i = kt - p0
                                    pp0, pp1 = pr[kt][qj]
                                    wdt = pp1 - pp0
                                    sl = slice(i * 512 + pp0, i * 512 + pp1)
                                    nc.vector.tensor_tensor(
                                        pt[:, sl], pt[:, sl],
                                        msk[:, moff[cl]:moff[cl] + wdt],
                                        op=mult)
                            for kt in pair:
                                i = kt - p0
                                c0, c1 = rng[kt][qj]
                                g = b * NKT + kt
                                vap = V_big[:, g * 256 + h * 128:
                                            g * 256 + (h + 1) * 128]
                                n_done[h] += 1
                                nc.tensor.matmul(
                                    ots[h][:, c0:c1], vap,
                                    pt[:, i * 512 + c0:i * 512 + c1],
                                    start=(n_done[h] == 1),
                                    stop=(n_done[h] == len(acts)))
                    for h in range(2):
                        # normalize: row 64 of ot = Z (sum of exp)
                        hs = slice(h * 64, (h + 1) * 64)
                        ot = ots[h]
                        z = zzp.tile([1, 512], f32, tag="z")
                        nc.vector.tensor_copy(z[:], ot[64:65, :])
                        rz = zzp.tile([1, 512], f32, tag="z")
                        nc.vector.reciprocal_approx_fast(rz[:], z[:])
                        rb = zbp.tile([64, 512], f32, tag="zb")
                        nc.gpsimd.partition_broadcast(rb[:], rz[:],
                                                      channels=64)
                        nc.vector.tensor_tensor(
                            OT_sb[hs, qlo:qlo + 512],
                            ot[0:64, :], rb[:], op=mult)
                    if not defer_oproj:
                        emit_oproj_qblock(b, qj)

            # ---- output projection (partial over local e-dims),
            # per q-block so outputs stream during attention. The two
            # PSUM->SBUF cast-copies split across DVE and ACT (Pool
            # cannot read PSUM in this backend) ----
            def emit_oproj_qblock(b, qj, tail=False):
                for g in range(b * 16 + qj * 4, b * 16 + (qj + 1) * 4):
                    osr = ostp.tile([128, D], f16, tag="ost")
                    for j in range(2):
                        # tail oprojs borrow the projection PSUM ring
                        # (projections are done by then) so they get
                        # double-buffered PSUM instead of fighting the
                        # live ots tiles in psO
                        if tail:
                            po = psA.tile([128, CH], f32, tag="proj")
                        else:
                            po = psO.tile([128, 512], f32, tag="ot")
                        nc.tensor.matmul(po[:],
                                         OT_sb[:, g * 128:(g + 1) * 128],
                                         woT_sb[:, j * 512:(j + 1) * 512],
                                         start=True, stop=True)
                        if j == 0:
                            nc.vector.tensor_copy(
                                osr[:, j * 512:(j + 1) * 512], po[:])
                        else:
                            nc.scalar.copy(
                                osr[:, j * 512:(j + 1) * 512], po[:])
                    nc.sync.dma_start(out=out.ap()[g * 128:(g + 1) * 128, :],
                                      in_=osr[:])

            # per-q-block interleave: chunk c feeds attention q-block c,
            # so projection matmuls act as PE filler while ACT runs exp.
            def emit_chunk3(c, first=False):
                for j, (name, src_r, dst) in enumerate(JT):
                    emit_proj_chunk(j, name, src_r, dst, c,
                                    split=(first and j == 0))
                    if first and j == 0:
                        emit_vz()
                    if first and j == 1:
                        emit_rest_consts()

            # batch 1 processes its smallest q-block (qj0, 4 k-tiles) LAST
            # so the final exp->AV->norm->oproj chain is as short as
            # possible; the deferred oprojs of qj2/qj3 fill the tail.
            for b in range(B):
                order = list(range(NQB)) if b == 0 else [1, 2, 3, 0]
                emitted = 0
                for qj in order:
                    # attention for this q-block may reach any k-tile the
                    # mask leaves active: emit every chunk it needs first
                    need = max([kt // 4 for kt in range(NKT)
                                if cls[kt][qj] != SKIP] + [qj])
                    while emitted <= need:
                        emit_chunk3(b * NQB + emitted,
                                    first=(b == 0 and emitted == 0))
                        emitted += 1
                    if b == 1 and qj == 0:
                        # qj2/qj3 oprojs (ready: their norms are done) are
                        # emitted before the final q-block so the static
                        # schedule uses them as PE filler during its exps
                        emit_oproj_qblock(1, 2, tail=True)
                        emit_oproj_qblock(1, 3, tail=True)
                    emit_attention(b, [qj],
                                   defer_oproj=(b == 1 and qj != 1))
                if b == 1:
                    emit_oproj_qblock(1, 0, tail=True)
                while emitted < NQB:
                    emit_chunk3(b * NQB + emitted)
                    emitted += 1

    nc.compile()
    return nc


_CACHE = {}


def kernel(q, k, v, mask, wq, bq, wk, bk, wv, bv, wo, bo):
    from concourse.bass_utils import run_bass_kernel_spmd

    q = np.ascontiguousarray(np.asarray(q, np.float32).reshape(BS, D))
    k = np.ascontiguousarray(np.asarray(k, np.float32).reshape(BS, D))
    v = np.ascontiguousarray(np.asarray(v, np.float32).reshape(BS, D))
    wq = np.asarray(wq, np.float32)
    wk = np.asarray(wk, np.float32)
    wv = np.asarray(wv, np.float32)
    wo = np.asarray(wo, np.float32)
    bq = np.asarray(bq, np.float32)
    bv = np.asarray(bv, np.float32)
    bo = np.asarray(bo, np.float32)

    qTf = np.ascontiguousarray(q.T.astype(np.float16))
    kTf = np.ascontiguousarray(k.T.astype(np.float16))
    vTf = np.ascontiguousarray(v.T.astype(np.float16))

    cls, rng, pr, pats = _classify_mask(mask)
    pat_widths = [p.shape[1] for p in pats]
    key = (tuple(tuple(r) for r in cls), tuple(tuple(r) for r in rng),
           tuple(tuple(r) for r in pr), tuple(pat_widths))
    if key not in _CACHE:
        _CACHE[key] = _build_program(cls, rng, pr, pat_widths)
    nc = _CACHE[key]

    if pats:
        masks_np = np.ascontiguousarray(
            np.concatenate(pats, axis=1).astype(np.float16))
    else:
        masks_np = np.zeros((128, 1), np.float16)

    def pack_w3(c):
        el = slice(c * EL, (c + 1) * EL)
        ws = []
        for w in (wq, wk, wv):
            wt = np.ascontiguousarray(w[el, :].T.astype(np.float16))
            ws.append(wt.reshape(NDT, 128, EL).transpose(1, 0, 2))
        return np.ascontiguousarray(
            np.stack(ws, axis=1).reshape(128, 3 * NDT * EL))

    in_maps = []
    for c in range(NC):
        el = slice(c * EL, (c + 1) * EL)
        m = {
            "qT": qTf, "kT": kTf, "vT": vTf,
            "w3": pack_w3(c),
            "woT": np.ascontiguousarray(wo[:, el].T.astype(np.float16)),
            "bq": np.ascontiguousarray(bq[el][:, None]),
            "masks": masks_np,
        }
        in_maps.append(m)

    res = run_bass_kernel_spmd(nc, in_maps, list(range(NC)))
    acc = res.results[0]["out"].astype(np.float32)
    for c in range(1, NC):
        acc = acc + res.results[c]["out"]
    # bo plus the folded V bias: softmax weights sum to 1 so the V bias
    # contributes wo @ bv to every output row
    acc = acc + (bo + wo @ bv)[None, :]
    return acc.reshape(B, S, D)
